# revision 21
# baseline (speedup 1.0000x reference)
"""DRNN-Char (4-layer dilated QRNN + decoder) Trainium2 kernel.

Sharding: data-parallel over batch. 16 batch rows across 8 cores = 2 rows/core.
Weights replicated. Each core computes its 2 rows fully on-chip.

Layout: activations are kept feature-major [feat, time] in SBUF so that
  - gate matmuls  Y^T = W^T @ X^T  put time on the PSUM free dim
  - the fo-pool recurrence maps onto DVE tensor_tensor_scan along the free dim
  - dilated layers use strided scan APs (stride = rate), no data movement

Layer 0 has no matmul at all: the host precomputes W0f = embT @ W0 and
gathers Y0 = W0f[x] per token (embedding lookup + layer-0 linear fused into
one table gather), uploading the pre-activation gates directly. They stream
into SBUF ahead of the scalar engine.

Gate math per layer (signs arranged so every elementwise op is cheap):
  u = tanh(z_pre + bz)          scalar engine, Tanh LUT, bf16 out
  f = sigmoid(f_pre + bf)       scalar engine, bf16 out
  so = sigmoid(o_pre + bo)      scalar engine, bf16 out
  zz = (f - 1) * u              gpsimd STT (frees the vector engine)
  e = scan(f, zz), init 0       DVE tensor_tensor_scan; e == -C exactly
  h' = e * so = -H              DVE tensor_tensor (bf16 packed -> 2x mode)
The -1 factor is folded into the next layer's weights on the host
(W <- -W for layers 1..3 and the decoder weight).

Matmuls accumulate k-outer/q-inner into a [128, 2048] PSUM tile (4 banks)
so consecutive matmuls share the stationary weight tile, and the scalar
engine drains each gate with a single 2048-wide activation.
"""

import numpy as np
import ml_dtypes

EMB = 256
HID = 512
LAYERS = 4
VOCAB = 256
B = 16
T = 2048
NCORES = 8
BC = B // NCORES          # batch rows per core
HCH = HID // 128          # hidden chunks
MCH = 3 * HCH             # m-chunks of the 3H gate output
NMM = 512                 # matmul moving free dim per instruction

_cache = {}


def _build():
    """Build + compile the SPMD bass program (cached across calls)."""
    if "nc" in _cache:
        return _cache["nc"]

    import concourse.bass as bass
    import concourse.mybir as mybir
    import concourse.tile as tile
    from concourse import bacc

    f32 = mybir.dt.float32
    bf16 = mybir.dt.bfloat16
    SIG = mybir.ActivationFunctionType.Sigmoid
    TANH = mybir.ActivationFunctionType.Tanh
    COPY = mybir.ActivationFunctionType.Copy
    MULT = mybir.AluOpType.mult
    ADD = mybir.AluOpType.add
    SUB = mybir.AluOpType.subtract

    nc = bacc.Bacc(
        "TRN2",
        target_bir_lowering=False,
        debug=False,
        enable_asserts=False,
        num_devices=NCORES,
    )

    # ---- DRAM parameters (per-core inputs prepared by the host) ----
    # y0: pre-activation layer-0 gates, host-gathered: [BC, MCH, 128, T]
    y0_d = nc.dram_tensor("y0", [BC, MCH, 128, T], bf16, kind="ExternalInput").ap()
    w_d = [None] + [
        nc.dram_tensor(f"w{i}", [4, 128, 3 * HID], bf16, kind="ExternalInput").ap()
        for i in range(1, LAYERS)
    ]
    wd_d = nc.dram_tensor("wd", [4, 128, VOCAB], bf16, kind="ExternalInput").ap()
    bias_d = nc.dram_tensor("bias", [LAYERS, 128, MCH], f32, kind="ExternalInput").ap()
    decb_d = nc.dram_tensor("decb", [1, VOCAB], bf16, kind="ExternalInput").ap()
    out_d = nc.dram_tensor("out", [BC, T, VOCAB], f32, kind="ExternalOutput").ap()

    with tile.TileContext(nc) as tc:
        with (
            tc.tile_pool(name="consts", bufs=1) as consts,
            tc.tile_pool(name="acts", bufs=1) as acts,
            tc.tile_pool(name="y0p", bufs=6) as y0p,
            tc.tile_pool(name="gf", bufs=3) as gf,
            tc.tile_pool(name="gu", bufs=3) as gu,
            tc.tile_pool(name="go", bufs=3) as go,
            tc.tile_pool(name="ge", bufs=3) as ge,
            tc.tile_pool(name="outs", bufs=4) as outs,
            tc.tile_pool(name="psum", bufs=2, space="PSUM") as psum,
            tc.tile_pool(name="zzp", bufs=1, space="PSUM") as zzp,
        ):
            # ---- resident tiles ----
            w_sb = [None] + [
                consts.tile([128, 4, 3 * HID], bf16, tag=f"w{i}", name=f"w{i}")
                for i in range(1, LAYERS)
            ]
            wd = consts.tile([128, 4, VOCAB], bf16, tag="wd", name="wd")
            bias = consts.tile([128, LAYERS, MCH], f32, tag="bias", name="bias")
            decb = consts.tile([1, VOCAB], bf16, tag="decb", name="decb")
            ones = consts.tile([1, 128], bf16, tag="ones", name="ones")

            # ping-pong activation buffers, [128, kchunk, T] bf16, per row
            xbuf = [acts.tile([128, 4, T], bf16, tag=f"x{r}", name=f"x{r}") for r in range(BC)]
            hbuf = [acts.tile([128, 4, T], bf16, tag=f"h{r}", name=f"h{r}") for r in range(BC)]

            # ---- input DMA ----
            for li in range(LAYERS):
                nc.gpsimd.dma_start(bias[:, li, :], bias_d[li])
            for k in range(4):
                nc.gpsimd.dma_start(w_sb[1][:, k, :], w_d[1][k])
            nc.gpsimd.dma_start(decb[:], decb_d[:])
            nc.gpsimd.memset(ones[:], 1.0)
            for i in range(2, LAYERS):
                for k in range(4):
                    nc.gpsimd.dma_start(w_sb[i][:, k, :], w_d[i][k])
            for k in range(4):
                nc.gpsimd.dma_start(wd[:, k, :], wd_d[k])

            # ---- QRNN layers ----
            for li in range(LAYERS):
                rate = 2 ** li
                Wt = w_sb[li]
                for r in range(BC):
                    xin, hout = xbuf[r], hbuf[r]
                    for h in range(HCH):
                        gt = {}
                        for gi, gname in enumerate(("z", "f", "o")):
                            m = gi * HCH + h
                            gpool = {"z": gu, "f": gf, "o": go}[gname]
                            # f stays f32: the DVE scan runs ~60% slower on
                            # bf16 operands (no fast 16-bit path, just
                            # conversion overhead). o is bf16 for gpsimd.
                            gdt = f32 if gname == "f" else bf16
                            g = gpool.tile([128, T], gdt, tag=gname, name=gname)
                            if li == 0:
                                # host-gathered pre-activations, stream from DRAM
                                ysb = y0p.tile([128, T], bf16, tag="y0", name="y0")
                                nc.sync.dma_start(ysb[:], y0_d[r, m])
                                nc.scalar.activation(
                                    g[:],
                                    ysb[:],
                                    TANH if gi == 0 else SIG,
                                    bias=bias[:, li, m : m + 1],
                                )
                            else:
                                # two [128,1024] psum tiles per gate (bufs=4 ->
                                # deeper tensor/scalar pipelining, 2-bank tiles)
                                for half in range(2):
                                    ps = psum.tile([128, T // 2], f32, tag="ps", name="ps")
                                    for k in range(4):
                                        for q in range(2):
                                            qq = half * 2 + q
                                            nc.tensor.matmul(
                                                ps[:, q * NMM : (q + 1) * NMM],
                                                lhsT=Wt[:, k, m * 128 : (m + 1) * 128],
                                                rhs=xin[:, k, qq * NMM : (qq + 1) * NMM],
                                                start=(k == 0),
                                                stop=(k == 3),
                                            )
                                    nc.scalar.activation(
                                        g[:, half * (T // 2) : (half + 1) * (T // 2)],
                                        ps[:],
                                        TANH if gi == 0 else SIG,
                                        bias=bias[:, li, m : m + 1],
                                    )
                            gt[gname] = g
                        # zz = (f - 1) * u, f32, staged through PSUM so the
                        # scan reads one operand per memory space (SBUF port
                        # contention with the concurrent gpsimd out-mult)
                        zz = zzp.tile([128, T], f32, tag="zz", name="zz")
                        nc.vector.scalar_tensor_tensor(
                            zz[:], gt["f"][:], 1.0, gt["z"][:], SUB, MULT
                        )
                        # e = scan(f, zz) along time, stride = rate, init 0 (= -C)
                        # all-f32: the DVE scan slows ~40-60% with any bf16
                        # operand (input or output), measured on hardware
                        e = ge.tile([128, T], f32, tag="e", name="e")
                        if rate <= 2:
                            # two chained time-halves per subsequence: lets the
                            # next layer's matmuls start on the first half
                            half = T // 2
                            for j in range(rate):
                                nc.vector.tensor_tensor_scan(
                                    e[:, j:half:rate],
                                    gt["f"][:, j:half:rate],
                                    zz[:, j:half:rate],
                                    initial=0.0, op0=MULT, op1=ADD,
                                )
                            nc.gpsimd.tensor_tensor(
                                hout[:, h, 0:half], e[:, 0:half],
                                gt["o"][:, 0:half], MULT,
                            )
                            for j in range(rate):
                                nc.vector.tensor_tensor_scan(
                                    e[:, half + j : T : rate],
                                    gt["f"][:, half + j : T : rate],
                                    zz[:, half + j : T : rate],
                                    initial=e[:, half - rate + j : half - rate + j + 1],
                                    op0=MULT, op1=ADD,
                                )
                            nc.gpsimd.tensor_tensor(
                                hout[:, h, half:T], e[:, half:T],
                                gt["o"][:, half:T], MULT,
                            )
                        else:
                            for j in range(rate):
                                sl = slice(j, T, rate)
                                nc.vector.tensor_tensor_scan(
                                    e[:, sl],
                                    gt["f"][:, sl],
                                    zz[:, sl],
                                    initial=0.0,
                                    op0=MULT,
                                    op1=ADD,
                                )
                            # h' = e * so  (= -H, bf16) on gpsimd
                            nc.gpsimd.tensor_tensor(
                                hout[:, h, :], e[:], gt["o"][:], MULT
                            )
                    xbuf[r], hbuf[r] = hbuf[r], xbuf[r]

            # ---- decoder: out[t, v] = H^T[:,t] . (-decW)[:, v] + decb ----
            for r in range(BC):
                xin = xbuf[r]
                for mt in range(T // 128):
                    psd = psum.tile([128, T // 2], f32, tag="ps", name="ps")
                    ps = psd[:, 0:VOCAB]
                    for k in range(4):
                        nc.tensor.matmul(
                            ps,
                            lhsT=xin[:, k, mt * 128 : (mt + 1) * 128],
                            rhs=wd[:, k, :],
                            start=(k == 0),
                            stop=False,
                        )
                    nc.tensor.matmul(
                        ps,
                        lhsT=ones[:],
                        rhs=decb[:],
                        start=False,
                        stop=True,
                    )
                    ot = outs.tile([128, VOCAB], f32, tag="ot", name="ot")
                    nc.scalar.activation(ot[:], ps, COPY)
                    nc.sync.dma_start(out_d[r, mt * 128 : (mt + 1) * 128, :], ot[:])

    nc.compile()
    _cache["nc"] = nc
    return nc


def _prep_inputs(inputs):
    """Host-side sharding + layout/dtype prep. Returns in_maps for 8 cores."""
    bf = ml_dtypes.bfloat16
    x = np.asarray(inputs["x"]).astype(np.int64)
    emb = np.asarray(inputs["emb"], dtype=np.float32)
    Ws = [np.asarray(inputs[f"W{i}"], dtype=np.float32) for i in range(LAYERS)]
    bs = [np.asarray(inputs[f"b{i}"], dtype=np.float32) for i in range(LAYERS)]
    decW = np.asarray(inputs["decW"], dtype=np.float32)
    decb = np.asarray(inputs["decb"], dtype=np.float32)

    # fused layer-0 table: W0f[v, :] = emb[v, :] @ W0, transposed to [3H, VOCAB]
    w0ft = (emb @ Ws[0]).T.astype(bf)  # [1536, 256]
    wscaled = [(-1.0 * Ws[i]).reshape(4, 128, 3 * HID).astype(bf) for i in range(1, LAYERS)]
    wd = (-1.0 * decW).reshape(4, 128, VOCAB).astype(bf)

    bias = np.zeros((LAYERS, 128, MCH), np.float32)
    for li in range(LAYERS):
        bias[li] = bs[li].reshape(MCH, 128).T  # [128, m]

    decbb = decb.reshape(1, VOCAB).astype(bf)

    in_maps = []
    for c in range(NCORES):
        # y0[r, m, p, t] = W0f[x[row], m*128+p] pre-activations, gathered on host
        y0 = np.empty((BC, MCH, 128, T), bf)
        for r in range(BC):
            y0[r] = w0ft[:, x[BC * c + r]].reshape(MCH, 128, T)
        in_maps.append(
            {
                "y0": y0,
                "w1": wscaled[0],
                "w2": wscaled[1],
                "w3": wscaled[2],
                "wd": wd,
                "bias": bias,
                "decb": decbb,
            }
        )
    return in_maps


def kernel(**inputs) -> np.ndarray:
    from concourse.bass_utils import run_bass_kernel_spmd

    try:  # reuse compiled NEFFs across kernel() invocations in one environment
        import jax, tempfile, os

        jax.config.update(
            "jax_compilation_cache_dir",
            os.environ.get("JAX_COMPILATION_CACHE_DIR")
            or os.path.join(tempfile.gettempdir(), "bass_jax_cache"),
        )
    except Exception:
        pass

    nc = _build()
    in_maps = _prep_inputs(inputs)
    res = run_bass_kernel_spmd(nc, in_maps, list(range(NCORES)))
    out = np.empty((B, T, VOCAB), np.float32)
    for c in range(NCORES):
        out[BC * c : BC * (c + 1)] = res.results[c]["out"]
    return out
